# revision 4
# baseline (speedup 1.0000x reference)
"""CaMoE block (LayerNorm -> per-expert squared-ReLU FFN with top-1 routing,
confidence-scaled combine, residual) on 8 Trainium2 NeuronCores.

Strategy (token-parallel with expert-grouped tiles):
  * Host: stable-sort tokens by winning expert, pad each expert group to a
    multiple of 128*8 so every core receives the SAME number of 128-token
    tiles per expert. This makes the SPMD program identical across cores
    while every 128-token tile has a single expert.
  * Device (per core): for each 128-token tile: LayerNorm (token-major),
    confidence sigmoid(h.wc+bc) and straight-through scale c/(c+1e-6);
    transpose h via the PE; then stream the expert's W1/W2 in H-chunks and
    run  kT = relu(W1^T h^T)^2  (PE + DVE) and  y += kT^T W2chunk  (PE),
    finally  out = y*scale + x  (DVE) and DMA out.
  * Host: scatter rows back to their original token positions.

gamma/beta of the LayerNorm are folded into W1/wc on the host (plus an
additive H-bias when beta != 0), so the device computes the pre-affine LN.
All matmuls run in bf16 with fp32 PSUM accumulation.
"""

import math
import os
from contextlib import ExitStack

import numpy as np

import concourse.bass as bass
import concourse.mybir as mybir
import concourse.tile as tile
from concourse.bass_utils import run_bass_kernel_spmd
from concourse.masks import make_identity
from concourse.tile import TileContext, ScopedClock

AF = mybir.ActivationFunctionType
OP = mybir.AluOpType
BF16 = mybir.dt.bfloat16
F32 = mybir.dt.float32
NP_BF16 = mybir.dt.np(BF16)

NCORES = 8
TILE = 128
HCHUNK = 512
LN_EPS = 1e-5

# ---------------------------------------------------------------------------
# Workarounds for the walrus build in this environment: it encodes at most
# ONE semaphore wait per instruction and cannot split multi-wait
# instructions itself ("Too many sync wait commands"). We (a) emit the
# TileContext tail-drain waits one-per-NoOp and (b) post-process the whole
# program to hoist excess waits onto same-engine NoOps.
# ---------------------------------------------------------------------------


def _patched_drain_and_barrier(self, tick_clock, wait_clock):
    probe = self.nc.sync.nop(nofuse=True)
    wait_clock.add_sem_waits(probe.ins, ScopedClock({None: tick_clock.global_clock}))
    si = probe.ins.sync_info
    waits = list(si.on_wait) if si is not None and si.on_wait else []
    if len(waits) > 1:
        probe.ins.sync_info = mybir.SyncInfo(on_wait=[waits[0]], on_update=[])
        for w in waits[1:]:
            n = self.nc.sync.nop(nofuse=True)
            n.ins.sync_info = mybir.SyncInfo(on_wait=[w], on_update=[])
    self.nc.sync.drain()
    self.nc.all_engine_barrier()
    assert self.sems is not None
    popped = self.nc._tile_sem_poison_stack.pop()
    assert popped is self._sem_poison
    self.nc.clear_and_free_semaphores(list(self.sems.allocated().values()))
    self.nc.all_engine_barrier()


TileContext._drain_and_barrier = _patched_drain_and_barrier


def _split_excess_waits(nc, max_waits: int = 1):
    for fn in nc.m.functions:
        for bb in fn.blocks:
            insts = list(bb.instructions)
            out = []
            changed = False
            for inst in insts:
                si = inst.sync_info
                waits = list(si.on_wait) if si is not None and si.on_wait else []
                if len(waits) > max_waits:
                    extra = waits[:-max_waits]
                    keep = waits[-max_waits:]
                    for j, w in enumerate(extra):
                        nop = mybir.InstNoOp(
                            name=f"{inst.name}-wsplit{j}", ins=[], outs=[]
                        )
                        nop.engine = inst.engine
                        nop.sync_info = mybir.SyncInfo(on_wait=[w], on_update=[])
                        out.append(nop)
                    inst.sync_info = mybir.SyncInfo(
                        on_wait=keep,
                        on_update=list(si.on_update) if si.on_update else [],
                    )
                    changed = True
                out.append(inst)
            if changed:
                bb.instructions = out


# ---------------------------------------------------------------------------
# Device program
# ---------------------------------------------------------------------------


def _build_program(C, H, M, S, passes, bc_fold, zero_bias):
    """Emit the SPMD Bass program. `passes` is a list of
    (slot, tile_offset, n_tiles<=2); every core runs the same program on its
    own data."""
    NKC = C // TILE          # K-tiles over C (8)
    NMH = HCHUNK // TILE     # M-tiles per H-chunk (4)
    NHC = H // HCHUNK        # H-chunks (8)
    NC2 = C // 512           # output column chunks (2)
    HN = H // TILE           # bias columns (32)

    nc = bass.Bass("TRN2", target_bir_lowering=False, debug=False)
    xc = nc.dram_tensor("xc", [M, C], F32, kind="ExternalInput").ap()
    w1b = nc.dram_tensor("w1b", [S, C, H], BF16, kind="ExternalInput").ap()
    w2b = nc.dram_tensor("w2b", [S, H, C], BF16, kind="ExternalInput").ap()
    wcb = nc.dram_tensor("wcb", [S, TILE, C], BF16, kind="ExternalInput").ap()
    if not zero_bias:
        b1b = nc.dram_tensor("b1b", [S, TILE, HN], F32, kind="ExternalInput").ap()
    yc = nc.dram_tensor("yc", [M, C], F32, kind="ExternalOutput").ap()

    with TileContext(nc) as tc, ExitStack() as ctx:
        cpool = ctx.enter_context(tc.tile_pool(name="const", bufs=1))
        ident = cpool.tile([TILE, TILE], BF16, tag="ident")
        make_identity(nc, ident[:])
        epsc = cpool.tile([TILE, 1], F32, tag="eps")
        nc.gpsimd.memset(epsc[:], LN_EPS)
        bccs = []
        for si in range(S):
            t = cpool.tile([TILE, 1], F32, tag=f"bc{si}")
            nc.gpsimd.memset(t[:], float(bc_fold[si]))
            bccs.append(t)

        wpool = ctx.enter_context(tc.tile_pool(name="w", bufs=3))
        spool = ctx.enter_context(tc.tile_pool(name="slot", bufs=2))
        xpool = ctx.enter_context(tc.tile_pool(name="x", bufs=6))
        hpool = ctx.enter_context(tc.tile_pool(name="h", bufs=3))
        htpool = ctx.enter_context(tc.tile_pool(name="ht", bufs=2))
        kpool = ctx.enter_context(tc.tile_pool(name="kt", bufs=4))
        opool = ctx.enter_context(tc.tile_pool(name="o", bufs=4))
        stpool = ctx.enter_context(tc.tile_pool(name="st", bufs=8))
        sqpool = ctx.enter_context(tc.tile_pool(name="sq", bufs=2))
        pps = ctx.enter_context(tc.tile_pool(name="pk", bufs=2, space="PSUM"))
        ppy = ctx.enter_context(tc.tile_pool(name="py", bufs=4, space="PSUM"))
        ppt = ctx.enter_context(tc.tile_pool(name="ptr", bufs=2, space="PSUM"))

        prev_slot = -1
        wcb_sb = None
        b1_sb = None
        for (si, tile_off, nt) in passes:
            ntok = TILE * nt
            if si != prev_slot:
                wcb_sb = spool.tile([TILE, C], BF16, tag="wcb")
                nc.sync.dma_start(wcb_sb[:], wcb[si])
                if not zero_bias:
                    b1_sb = spool.tile([TILE, HN], F32, tag="b1")
                    nc.sync.dma_start(b1_sb[:], b1b[si])
                prev_slot = si

            x_t = []
            s_t = []
            hT = htpool.tile([TILE, NKC, ntok], BF16, tag="hT")
            for t in range(nt):
                row0 = (tile_off + t) * TILE
                xt = xpool.tile([TILE, C], F32, tag="x")
                x_t.append(xt)
                nc.sync.dma_start(xt[:], xc[row0 : row0 + TILE, :])

                nsum = stpool.tile([TILE, 1], F32, tag="nsum")
                nc.vector.reduce_sum(
                    nsum[:], xt[:], axis=mybir.AxisListType.X, negate=True
                )
                negmu = stpool.tile([TILE, 1], F32, tag="negmu")
                nc.scalar.mul(negmu[:], nsum[:], 1.0 / C)
                sq = sqpool.tile([TILE, C], F32, tag="sq")
                ssq = stpool.tile([TILE, 1], F32, tag="ssq")
                nc.scalar.activation(
                    sq[:], xt[:], AF.Square, bias=negmu[:], scale=1.0,
                    accum_out=ssq[:],
                )
                std = stpool.tile([TILE, 1], F32, tag="std")
                nc.scalar.activation(
                    std[:], ssq[:], AF.Sqrt, bias=epsc[:], scale=1.0 / C
                )
                rs = stpool.tile([TILE, 1], F32, tag="rs")
                nc.vector.reciprocal(rs[:], std[:])
                nmrs = stpool.tile([TILE, 1], F32, tag="nmrs")
                nc.vector.tensor_mul(nmrs[:], negmu[:], rs[:])
                ht_ = hpool.tile([TILE, C], BF16, tag="h")
                nc.scalar.activation(
                    ht_[:], xt[:], AF.Identity, bias=nmrs[:], scale=rs[:]
                )

                # confidence -> straight-through scale
                prod = hpool.tile([TILE, C], BF16, tag="prod")
                cdot = stpool.tile([TILE, 1], F32, tag="cdot")
                nc.vector.scalar_tensor_tensor(
                    prod[:], ht_[:], 1.0, wcb_sb[:], op0=OP.mult, op1=OP.mult,
                    accum_out=cdot[:],
                )
                conf = stpool.tile([TILE, 1], F32, tag="conf")
                nc.scalar.activation(
                    conf[:], cdot[:], AF.Sigmoid, bias=bccs[si][:], scale=1.0
                )
                cpe = stpool.tile([TILE, 1], F32, tag="cpe")
                nc.vector.tensor_scalar_add(cpe[:], conf[:], 1e-6)
                rc = stpool.tile([TILE, 1], F32, tag="rc")
                nc.vector.reciprocal(rc[:], cpe[:])
                sc = stpool.tile([TILE, 1], F32, tag="sc")
                nc.vector.tensor_mul(sc[:], conf[:], rc[:])
                s_t.append(sc)

                # h^T tiles for the matmuls
                for kc in range(NKC):
                    pt = ppt.tile([TILE, TILE], BF16, tag="ptr")
                    nc.tensor.transpose(
                        pt[:], ht_[:, kc * TILE : (kc + 1) * TILE], ident[:]
                    )
                    nc.vector.tensor_copy(
                        hT[:, kc, t * TILE : (t + 1) * TILE], pt[:]
                    )

            ys = [
                ppy.tile([TILE, 512], F32, tag="py", name=f"ys{i}")
                for i in range(nt * NC2)
            ]
            for hc in range(NHC):
                w1t = wpool.tile([TILE, NKC, HCHUNK], BF16, tag="w1")
                nc.sync.dma_start(
                    w1t[:],
                    w1b[si, :, hc * HCHUNK : (hc + 1) * HCHUNK].rearrange(
                        "(kc p) h -> p kc h", p=TILE
                    ),
                )
                w2t = wpool.tile([TILE, NMH, C], BF16, tag="w2")
                nc.sync.dma_start(
                    w2t[:],
                    w2b[si, hc * HCHUNK : (hc + 1) * HCHUNK, :].rearrange(
                        "(kh p) c -> p kh c", p=TILE
                    ),
                )
                for mh in range(NMH):
                    pk = pps.tile([TILE, ntok], F32, tag="pk")
                    for kc in range(NKC):
                        nc.tensor.matmul(
                            pk[:],
                            w1t[:, kc, mh * TILE : (mh + 1) * TILE],
                            hT[:, kc, :],
                            start=(kc == 0),
                            stop=(kc == NKC - 1),
                        )
                    kt = kpool.tile([TILE, ntok], BF16, tag="kt")
                    kr = kpool.tile([TILE, ntok], BF16, tag="kr")
                    if zero_bias:
                        bias_ap = 0.0
                    else:
                        col = hc * NMH + mh
                        bias_ap = b1_sb[:, col : col + 1]
                    nc.scalar.activation(
                        kr[:], pk[:], AF.Relu, bias=bias_ap, scale=1.0
                    )
                    nc.vector.tensor_mul(kt[:], kr[:], kr[:])
                    for t in range(nt):
                        for ncx in range(NC2):
                            nc.tensor.matmul(
                                ys[t * NC2 + ncx][:],
                                kt[:, t * TILE : (t + 1) * TILE],
                                w2t[:, mh, ncx * 512 : (ncx + 1) * 512],
                                start=(hc == 0 and mh == 0),
                                stop=(hc == NHC - 1 and mh == NMH - 1),
                            )
            for t in range(nt):
                row0 = (tile_off + t) * TILE
                ot = opool.tile([TILE, C], F32, tag="o")
                for ncx in range(NC2):
                    nc.vector.scalar_tensor_tensor(
                        ot[:, ncx * 512 : (ncx + 1) * 512],
                        ys[t * NC2 + ncx][:],
                        s_t[t][:],
                        x_t[t][:, ncx * 512 : (ncx + 1) * 512],
                        op0=OP.mult,
                        op1=OP.add,
                    )
                nc.sync.dma_start(yc[row0 : row0 + TILE, :], ot[:])

    _split_excess_waits(nc, 1)
    return nc


# ---------------------------------------------------------------------------
# Host-side dispatch
# ---------------------------------------------------------------------------


def _prepare(x, winners, gamma, beta, w1, w2, wc, bc):
    x = np.ascontiguousarray(np.asarray(x, dtype=np.float32))
    winners = np.asarray(winners).reshape(-1).astype(np.int64)
    gamma = np.asarray(gamma, dtype=np.float32)
    beta = np.asarray(beta, dtype=np.float32)
    w1 = np.asarray(w1, dtype=np.float32)
    w2 = np.asarray(w2, dtype=np.float32)
    wc = np.asarray(wc, dtype=np.float32)
    bc = np.asarray(bc, dtype=np.float32)

    B, T, C = x.shape
    E, _, H = w1.shape
    N = B * T
    xf = x.reshape(N, C)

    order = np.argsort(winners, kind="stable")
    counts = np.bincount(winners, minlength=E)

    slots = [e for e in range(E) if counts[e] > 0]
    S = len(slots)
    grain = TILE * NCORES

    per_core_idx = [[] for _ in range(NCORES)]
    passes = []
    pos = 0
    tile_off = 0
    for si, e in enumerate(slots):
        n_e = int(counts[e])
        m_e = int(math.ceil(n_e / grain))
        padded = np.full(m_e * grain, -1, dtype=np.int64)
        padded[:n_e] = order[pos : pos + n_e]
        pos += n_e
        resh = padded.reshape(m_e, NCORES, TILE)
        for c in range(NCORES):
            per_core_idx[c].append(resh[:, c, :].reshape(-1))
        j = 0
        while j < m_e:
            nt = min(2, m_e - j)
            passes.append((si, tile_off + j, nt))
            j += nt
        tile_off += m_e
    per_core_idx = [np.concatenate(lst) for lst in per_core_idx]
    M = per_core_idx[0].size

    # fold gamma/beta
    w1f = (w1[slots] * gamma[None, :, None]).astype(NP_BF16)
    w2f = w2[slots].astype(NP_BF16)
    wcf = (wc[slots] * gamma[None, :]).astype(NP_BF16)
    wcb = np.ascontiguousarray(
        np.broadcast_to(wcf[:, None, :], (S, TILE, C))
    )
    zero_bias = bool(np.all(beta == 0.0))
    bc_fold = [float(bc[e] + float(beta @ wc[e])) for e in slots]
    b1b = None
    if not zero_bias:
        b1 = np.einsum("c,sch->sh", beta, w1[slots])
        b1b = np.ascontiguousarray(
            b1.reshape(S, H // TILE, TILE).transpose(0, 2, 1)
        ).astype(np.float32)

    in_maps = []
    for c in range(NCORES):
        idx = per_core_idx[c]
        xcrows = np.zeros((M, C), dtype=np.float32)
        valid = idx >= 0
        xcrows[valid] = xf[idx[valid]]
        m = {"xc": xcrows, "w1b": w1f, "w2b": w2f, "wcb": wcb}
        if not zero_bias:
            m["b1b"] = b1b
        in_maps.append(m)

    meta = dict(
        B=B, T=T, C=C, H=H, N=N, M=M, S=S, passes=passes,
        bc_fold=bc_fold, zero_bias=zero_bias, per_core_idx=per_core_idx,
        xf=xf,
    )
    return in_maps, meta


def _assemble(results, meta):
    N, C = meta["N"], meta["C"]
    out = np.empty((N, C), dtype=np.float32)
    seen = np.zeros(N, dtype=bool)
    for c in range(NCORES):
        idx = meta["per_core_idx"][c]
        valid = idx >= 0
        out[idx[valid]] = results[c]["yc"][valid]
        seen[idx[valid]] = True
    assert seen.all()
    return out.reshape(meta["B"], meta["T"], C)


def kernel_with_results(x, winners, gamma, beta, w1, w2, wc, bc, **run_kwargs):
    in_maps, meta = _prepare(x, winners, gamma, beta, w1, w2, wc, bc)
    nc = _build_program(
        meta["C"], meta["H"], meta["M"], meta["S"], meta["passes"],
        meta["bc_fold"], meta["zero_bias"],
    )
    res = run_bass_kernel_spmd(nc, in_maps, core_ids=list(range(NCORES)), **run_kwargs)
    return _assemble(res.results, meta), res


def kernel(x, winners, gamma, beta, w1, w2, wc, bc):
    out, _ = kernel_with_results(x, winners, gamma, beta, w1, w2, wc, bc)
    return out


# revision 10
# speedup vs baseline: 1.0448x; 1.0448x over previous
"""CaMoE block (LayerNorm -> per-expert squared-ReLU FFN with top-1 routing,
confidence-scaled combine, residual) on 8 Trainium2 NeuronCores.

Strategy (token-parallel with expert-grouped tiles):
  * Host: stable-sort tokens by winning expert, pad each expert group to a
    multiple of 128*8 so every core receives the SAME number of 128-token
    tiles per expert. This makes the SPMD program identical across cores
    while every 128-token tile has a single expert.
  * Device (per core): for each 128-token tile: LayerNorm (token-major),
    confidence sigmoid(h.wc+bc) and straight-through scale c/(c+1e-6);
    transpose h via the PE; then stream the expert's W1/W2 in H-chunks and
    run  kT = relu(W1^T h^T)^2  (PE + DVE) and  y += kT^T W2chunk  (PE),
    finally  out = y*scale + x  (DVE) and DMA out.
  * Host: scatter rows back to their original token positions.

gamma/beta of the LayerNorm are folded into W1/wc on the host (plus an
additive H-bias when beta != 0), so the device computes the pre-affine LN.
All matmuls run in bf16 with fp32 PSUM accumulation.
"""

import math
import os
from contextlib import ExitStack

import numpy as np

import concourse.bass as bass
import concourse.mybir as mybir
import concourse.tile as tile
from concourse.bass_utils import run_bass_kernel_spmd
from concourse.masks import make_identity
from concourse.tile import TileContext, ScopedClock

AF = mybir.ActivationFunctionType
OP = mybir.AluOpType
BF16 = mybir.dt.bfloat16
F32 = mybir.dt.float32
NP_BF16 = mybir.dt.np(BF16)

NCORES = 8
TILE = 128
HCHUNK = 512
LN_EPS = 1e-5

# ---------------------------------------------------------------------------
# Workarounds for the walrus build in this environment: it encodes at most
# ONE semaphore wait per instruction and cannot split multi-wait
# instructions itself ("Too many sync wait commands"). We (a) emit the
# TileContext tail-drain waits one-per-NoOp and (b) post-process the whole
# program to hoist excess waits onto same-engine NoOps.
# ---------------------------------------------------------------------------


def _patched_drain_and_barrier(self, tick_clock, wait_clock):
    probe = self.nc.sync.nop(nofuse=True)
    wait_clock.add_sem_waits(probe.ins, ScopedClock({None: tick_clock.global_clock}))
    si = probe.ins.sync_info
    waits = list(si.on_wait) if si is not None and si.on_wait else []
    if len(waits) > 1:
        probe.ins.sync_info = mybir.SyncInfo(on_wait=[waits[0]], on_update=[])
        for w in waits[1:]:
            n = self.nc.sync.nop(nofuse=True)
            n.ins.sync_info = mybir.SyncInfo(on_wait=[w], on_update=[])
    self.nc.sync.drain()
    self.nc.all_engine_barrier()
    assert self.sems is not None
    popped = self.nc._tile_sem_poison_stack.pop()
    assert popped is self._sem_poison
    self.nc.clear_and_free_semaphores(list(self.sems.allocated().values()))
    self.nc.all_engine_barrier()


TileContext._drain_and_barrier = _patched_drain_and_barrier


def _split_excess_waits(nc, max_waits: int = 1):
    for fn in nc.m.functions:
        for bb in fn.blocks:
            insts = list(bb.instructions)
            out = []
            changed = False
            for inst in insts:
                si = inst.sync_info
                waits = list(si.on_wait) if si is not None and si.on_wait else []
                if len(waits) > max_waits:
                    extra = waits[:-max_waits]
                    keep = waits[-max_waits:]
                    for j, w in enumerate(extra):
                        nop = mybir.InstNoOp(
                            name=f"{inst.name}-wsplit{j}", ins=[], outs=[]
                        )
                        nop.engine = inst.engine
                        nop.sync_info = mybir.SyncInfo(on_wait=[w], on_update=[])
                        out.append(nop)
                    inst.sync_info = mybir.SyncInfo(
                        on_wait=keep,
                        on_update=list(si.on_update) if si.on_update else [],
                    )
                    changed = True
                out.append(inst)
            if changed:
                bb.instructions = out


# ---------------------------------------------------------------------------
# Device program
# ---------------------------------------------------------------------------


def _build_program(C, H, M, S, passes, bc_fold, zero_bias):
    """Emit the SPMD Bass program. `passes` is a list of
    (slot, tile_offset, n_tiles<=2); every core runs the same program on its
    own data."""
    NKC = C // TILE          # K-tiles over C (8)
    NMH = HCHUNK // TILE     # M-tiles per H-chunk (4)
    NHC = H // HCHUNK        # H-chunks (8)
    NC2 = C // 512           # output column chunks (2)
    HN = H // TILE           # bias columns (32)

    WCOLS = NKC * HCHUNK + NMH * C  # w1-part then w2-part, tile-contiguous

    nc = bass.Bass("TRN2", target_bir_lowering=False, debug=False)
    xc = nc.dram_tensor("xc", [M, C], F32, kind="ExternalInput").ap()
    wr = nc.dram_tensor("wr", [S, NHC, TILE, WCOLS], BF16, kind="ExternalInput").ap()
    wcb = nc.dram_tensor("wcb", [S, TILE, C], BF16, kind="ExternalInput").ap()
    if not zero_bias:
        b1b = nc.dram_tensor("b1b", [S, TILE, HN], F32, kind="ExternalInput").ap()
    yc = nc.dram_tensor("yc", [M, C], F32, kind="ExternalOutput").ap()

    with TileContext(nc) as tc, ExitStack() as ctx:
        cpool = ctx.enter_context(tc.tile_pool(name="const", bufs=1))
        ident = cpool.tile([TILE, TILE], BF16, tag="ident")
        make_identity(nc, ident[:])
        epsc = cpool.tile([TILE, 1], F32, tag="eps")
        nc.gpsimd.memset(epsc[:], LN_EPS)
        bccs = []
        for si in range(S):
            t = cpool.tile([TILE, 1], F32, tag=f"bc{si}")
            nc.gpsimd.memset(t[:], float(bc_fold[si]))
            bccs.append(t)

        wpool = ctx.enter_context(tc.tile_pool(name="w", bufs=4))
        spool = ctx.enter_context(tc.tile_pool(name="slot", bufs=2))
        xpool = ctx.enter_context(tc.tile_pool(name="x", bufs=6))
        hpool = ctx.enter_context(tc.tile_pool(name="h", bufs=3))
        htpool = ctx.enter_context(tc.tile_pool(name="ht", bufs=2))
        kpool = ctx.enter_context(tc.tile_pool(name="kt", bufs=4))
        opool = ctx.enter_context(tc.tile_pool(name="o", bufs=4))
        stpool = ctx.enter_context(tc.tile_pool(name="st", bufs=8))
        sqpool = ctx.enter_context(tc.tile_pool(name="sq", bufs=2))
        pps = ctx.enter_context(tc.tile_pool(name="pk", bufs=2, space="PSUM"))
        ppy = ctx.enter_context(tc.tile_pool(name="py", bufs=4, space="PSUM"))
        ppt = ctx.enter_context(tc.tile_pool(name="ptr", bufs=2, space="PSUM"))

        prev_slot = -1
        wcb_sb = None
        b1_sb = None
        for (si, tile_off, nt) in passes:
            ntok = TILE * nt
            if si != prev_slot:
                wcb_sb = spool.tile([TILE, C], BF16, tag="wcb")
                nc.sync.dma_start(wcb_sb[:], wcb[si])
                if not zero_bias:
                    b1_sb = spool.tile([TILE, HN], F32, tag="b1")
                    nc.sync.dma_start(b1_sb[:], b1b[si])
                prev_slot = si

            x_t = []
            s_t = []
            hT = htpool.tile([TILE, NKC, ntok], BF16, tag="hT")
            for t in range(nt):
                row0 = (tile_off + t) * TILE
                xt = xpool.tile([TILE, C], F32, tag="x")
                x_t.append(xt)
                nc.sync.dma_start(xt[:], xc[row0 : row0 + TILE, :])

                nsum = stpool.tile([TILE, 1], F32, tag="nsum")
                nc.vector.reduce_sum(
                    nsum[:], xt[:], axis=mybir.AxisListType.X, negate=True
                )
                negmu = stpool.tile([TILE, 1], F32, tag="negmu")
                nc.scalar.mul(negmu[:], nsum[:], 1.0 / C)
                sq = sqpool.tile([TILE, C], F32, tag="sq")
                ssq = stpool.tile([TILE, 1], F32, tag="ssq")
                nc.scalar.activation(
                    sq[:], xt[:], AF.Square, bias=negmu[:], scale=1.0,
                    accum_out=ssq[:],
                )
                std = stpool.tile([TILE, 1], F32, tag="std")
                nc.scalar.activation(
                    std[:], ssq[:], AF.Sqrt, bias=epsc[:], scale=1.0 / C
                )
                rs = stpool.tile([TILE, 1], F32, tag="rs")
                nc.vector.reciprocal(rs[:], std[:])
                nmrs = stpool.tile([TILE, 1], F32, tag="nmrs")
                nc.vector.tensor_mul(nmrs[:], negmu[:], rs[:])
                ht_ = hpool.tile([TILE, C], BF16, tag="h")
                nc.scalar.activation(
                    ht_[:], xt[:], AF.Identity, bias=nmrs[:], scale=rs[:]
                )

                # confidence -> straight-through scale
                prod = hpool.tile([TILE, C], BF16, tag="prod")
                cdot = stpool.tile([TILE, 1], F32, tag="cdot")
                nc.vector.scalar_tensor_tensor(
                    prod[:], ht_[:], 1.0, wcb_sb[:], op0=OP.mult, op1=OP.mult,
                    accum_out=cdot[:],
                )
                conf = stpool.tile([TILE, 1], F32, tag="conf")
                nc.scalar.activation(
                    conf[:], cdot[:], AF.Sigmoid, bias=bccs[si][:], scale=1.0
                )
                cpe = stpool.tile([TILE, 1], F32, tag="cpe")
                nc.vector.tensor_scalar_add(cpe[:], conf[:], 1e-6)
                rc = stpool.tile([TILE, 1], F32, tag="rc")
                nc.vector.reciprocal(rc[:], cpe[:])
                sc = stpool.tile([TILE, 1], F32, tag="sc")
                nc.vector.tensor_mul(sc[:], conf[:], rc[:])
                s_t.append(sc)

                # h^T tiles for the matmuls
                for kc in range(NKC):
                    pt = ppt.tile([TILE, TILE], BF16, tag="ptr")
                    nc.tensor.transpose(
                        pt[:], ht_[:, kc * TILE : (kc + 1) * TILE], ident[:]
                    )
                    nc.vector.tensor_copy(
                        hT[:, kc, t * TILE : (t + 1) * TILE], pt[:]
                    )

            ys = [
                ppy.tile([TILE, 512], F32, tag="py", name=f"ys{i}")
                for i in range(nt * NC2)
            ]
            for hc in range(NHC):
                wt = wpool.tile([TILE, WCOLS], BF16, tag="w")
                nc.sync.dma_start(wt[:], wr[si, hc])
                for mh in range(NMH):
                    pk = pps.tile([TILE, ntok], F32, tag="pk")
                    for kc in range(NKC):
                        nc.tensor.matmul(
                            pk[:],
                            wt[:, kc * HCHUNK + mh * TILE : kc * HCHUNK + (mh + 1) * TILE],
                            hT[:, kc, :],
                            start=(kc == 0),
                            stop=(kc == NKC - 1),
                        )
                    kt = kpool.tile([TILE, ntok], BF16, tag="kt")
                    kr = kpool.tile([TILE, ntok], BF16, tag="kr")
                    if zero_bias:
                        bias_ap = 0.0
                    else:
                        col = hc * NMH + mh
                        bias_ap = b1_sb[:, col : col + 1]
                    nc.scalar.activation(
                        kr[:], pk[:], AF.Relu, bias=bias_ap, scale=1.0
                    )
                    nc.vector.tensor_mul(kt[:], kr[:], kr[:])
                    w2base = NKC * HCHUNK + mh * C
                    for t in range(nt):
                        for ncx in range(NC2):
                            nc.tensor.matmul(
                                ys[t * NC2 + ncx][:],
                                kt[:, t * TILE : (t + 1) * TILE],
                                wt[:, w2base + ncx * 512 : w2base + (ncx + 1) * 512],
                                start=(hc == 0 and mh == 0),
                                stop=(hc == NHC - 1 and mh == NMH - 1),
                            )
            for t in range(nt):
                row0 = (tile_off + t) * TILE
                ot = opool.tile([TILE, C], F32, tag="o")
                for ncx in range(NC2):
                    nc.vector.scalar_tensor_tensor(
                        ot[:, ncx * 512 : (ncx + 1) * 512],
                        ys[t * NC2 + ncx][:],
                        s_t[t][:],
                        x_t[t][:, ncx * 512 : (ncx + 1) * 512],
                        op0=OP.mult,
                        op1=OP.add,
                    )
                nc.sync.dma_start(yc[row0 : row0 + TILE, :], ot[:])

    _split_excess_waits(nc, 1)
    return nc


# ---------------------------------------------------------------------------
# Host-side dispatch
# ---------------------------------------------------------------------------


def _prepare(x, winners, gamma, beta, w1, w2, wc, bc):
    x = np.ascontiguousarray(np.asarray(x, dtype=np.float32))
    winners = np.asarray(winners).reshape(-1).astype(np.int64)
    gamma = np.asarray(gamma, dtype=np.float32)
    beta = np.asarray(beta, dtype=np.float32)
    w1 = np.asarray(w1, dtype=np.float32)
    w2 = np.asarray(w2, dtype=np.float32)
    wc = np.asarray(wc, dtype=np.float32)
    bc = np.asarray(bc, dtype=np.float32)

    B, T, C = x.shape
    E, _, H = w1.shape
    N = B * T
    xf = x.reshape(N, C)

    order = np.argsort(winners, kind="stable")
    counts = np.bincount(winners, minlength=E)

    slots = [e for e in range(E) if counts[e] > 0]
    S = len(slots)
    grain = TILE * NCORES

    per_core_idx = [[] for _ in range(NCORES)]
    passes = []
    pos = 0
    tile_off = 0
    for si, e in enumerate(slots):
        n_e = int(counts[e])
        m_e = int(math.ceil(n_e / grain))
        padded = np.full(m_e * grain, -1, dtype=np.int64)
        padded[:n_e] = order[pos : pos + n_e]
        pos += n_e
        resh = padded.reshape(m_e, NCORES, TILE)
        for c in range(NCORES):
            per_core_idx[c].append(resh[:, c, :].reshape(-1))
        j = 0
        while j < m_e:
            nt = min(2, m_e - j)
            passes.append((si, tile_off + j, nt))
            j += nt
        tile_off += m_e
    per_core_idx = [np.concatenate(lst) for lst in per_core_idx]
    M = per_core_idx[0].size

    # fold gamma/beta
    NKC = C // TILE
    NMH = HCHUNK // TILE
    NHC = H // HCHUNK
    w1f = (w1[slots] * gamma[None, :, None]).astype(NP_BF16)
    w2f = w2[slots].astype(NP_BF16)
    # re-layout weights into the exact SBUF tile order so each (slot, hchunk)
    # is ONE contiguous [128, WCOLS] DMA (16KB per partition row)
    w1part = (
        w1f.reshape(S, NKC, TILE, NHC, HCHUNK)
        .transpose(0, 3, 2, 1, 4)
        .reshape(S, NHC, TILE, NKC * HCHUNK)
    )
    w2part = (
        w2f.reshape(S, NHC, NMH, TILE, C)
        .transpose(0, 1, 3, 2, 4)
        .reshape(S, NHC, TILE, NMH * C)
    )
    wrearr = np.ascontiguousarray(np.concatenate([w1part, w2part], axis=3))
    wcf = (wc[slots] * gamma[None, :]).astype(NP_BF16)
    wcb = np.ascontiguousarray(
        np.broadcast_to(wcf[:, None, :], (S, TILE, C))
    )
    zero_bias = bool(np.all(beta == 0.0))
    bc_fold = [float(bc[e] + float(beta @ wc[e])) for e in slots]
    b1b = None
    if not zero_bias:
        b1 = np.einsum("c,sch->sh", beta, w1[slots])
        b1b = np.ascontiguousarray(
            b1.reshape(S, H // TILE, TILE).transpose(0, 2, 1)
        ).astype(np.float32)

    in_maps = []
    for c in range(NCORES):
        idx = per_core_idx[c]
        xcrows = np.zeros((M, C), dtype=np.float32)
        valid = idx >= 0
        xcrows[valid] = xf[idx[valid]]
        m = {"xc": xcrows, "wr": wrearr, "wcb": wcb}
        if not zero_bias:
            m["b1b"] = b1b
        in_maps.append(m)

    meta = dict(
        B=B, T=T, C=C, H=H, N=N, M=M, S=S, passes=passes,
        bc_fold=bc_fold, zero_bias=zero_bias, per_core_idx=per_core_idx,
        xf=xf,
    )
    return in_maps, meta


def _assemble(results, meta):
    N, C = meta["N"], meta["C"]
    out = np.empty((N, C), dtype=np.float32)
    seen = np.zeros(N, dtype=bool)
    for c in range(NCORES):
        idx = meta["per_core_idx"][c]
        valid = idx >= 0
        out[idx[valid]] = results[c]["yc"][valid]
        seen[idx[valid]] = True
    assert seen.all()
    return out.reshape(meta["B"], meta["T"], C)


def kernel_with_results(x, winners, gamma, beta, w1, w2, wc, bc, **run_kwargs):
    in_maps, meta = _prepare(x, winners, gamma, beta, w1, w2, wc, bc)
    nc = _build_program(
        meta["C"], meta["H"], meta["M"], meta["S"], meta["passes"],
        meta["bc_fold"], meta["zero_bias"],
    )
    res = run_bass_kernel_spmd(nc, in_maps, core_ids=list(range(NCORES)), **run_kwargs)
    return _assemble(res.results, meta), res


def kernel(x, winners, gamma, beta, w1, w2, wc, bc):
    out, _ = kernel_with_results(x, winners, gamma, beta, w1, w2, wc, bc)
    return out


# revision 12
# speedup vs baseline: 1.0541x; 1.0089x over previous
"""CaMoE block (LayerNorm -> per-expert squared-ReLU FFN with top-1 routing,
confidence-scaled combine, residual) on 8 Trainium2 NeuronCores.

Strategy (token-parallel with expert-grouped tiles):
  * Host: stable-sort tokens by winning expert, pad each expert group to a
    multiple of 128*8 so every core receives the SAME number of 128-token
    tiles per expert. This makes the SPMD program identical across cores
    while every 128-token tile has a single expert.
  * Device (per core): for each 128-token tile: LayerNorm (token-major),
    confidence sigmoid(h.wc+bc) and straight-through scale c/(c+1e-6);
    transpose h via the PE; then stream the expert's W1/W2 in H-chunks and
    run  kT = relu(W1^T h^T)^2  (PE + DVE) and  y += kT^T W2chunk  (PE),
    finally  out = y*scale + x  (DVE) and DMA out.
  * Host: scatter rows back to their original token positions.

gamma/beta of the LayerNorm are folded into W1/wc on the host (plus an
additive H-bias when beta != 0), so the device computes the pre-affine LN.
All matmuls run in bf16 with fp32 PSUM accumulation.
"""

import math
import os
from contextlib import ExitStack

import numpy as np

import concourse.bass as bass
import concourse.mybir as mybir
import concourse.tile as tile
from concourse.bass_utils import run_bass_kernel_spmd
from concourse.masks import make_identity
from concourse.tile import TileContext, ScopedClock

AF = mybir.ActivationFunctionType
OP = mybir.AluOpType
BF16 = mybir.dt.bfloat16
F32 = mybir.dt.float32
NP_BF16 = mybir.dt.np(BF16)

NCORES = 8
TILE = 128
HCHUNK = 512
LN_EPS = 1e-5

# ---------------------------------------------------------------------------
# Workarounds for the walrus build in this environment: it encodes at most
# ONE semaphore wait per instruction and cannot split multi-wait
# instructions itself ("Too many sync wait commands"). We (a) emit the
# TileContext tail-drain waits one-per-NoOp and (b) post-process the whole
# program to hoist excess waits onto same-engine NoOps.
# ---------------------------------------------------------------------------


def _patched_drain_and_barrier(self, tick_clock, wait_clock):
    probe = self.nc.sync.nop(nofuse=True)
    wait_clock.add_sem_waits(probe.ins, ScopedClock({None: tick_clock.global_clock}))
    si = probe.ins.sync_info
    waits = list(si.on_wait) if si is not None and si.on_wait else []
    if len(waits) > 1:
        probe.ins.sync_info = mybir.SyncInfo(on_wait=[waits[0]], on_update=[])
        for w in waits[1:]:
            n = self.nc.sync.nop(nofuse=True)
            n.ins.sync_info = mybir.SyncInfo(on_wait=[w], on_update=[])
    self.nc.sync.drain()
    self.nc.all_engine_barrier()
    assert self.sems is not None
    popped = self.nc._tile_sem_poison_stack.pop()
    assert popped is self._sem_poison
    self.nc.clear_and_free_semaphores(list(self.sems.allocated().values()))
    self.nc.all_engine_barrier()


TileContext._drain_and_barrier = _patched_drain_and_barrier


def _split_excess_waits(nc, max_waits: int = 1):
    for fn in nc.m.functions:
        for bb in fn.blocks:
            insts = list(bb.instructions)
            out = []
            changed = False
            for inst in insts:
                si = inst.sync_info
                waits = list(si.on_wait) if si is not None and si.on_wait else []
                if len(waits) > max_waits:
                    extra = waits[:-max_waits]
                    keep = waits[-max_waits:]
                    for j, w in enumerate(extra):
                        nop = mybir.InstNoOp(
                            name=f"{inst.name}-wsplit{j}", ins=[], outs=[]
                        )
                        nop.engine = inst.engine
                        nop.sync_info = mybir.SyncInfo(on_wait=[w], on_update=[])
                        out.append(nop)
                    inst.sync_info = mybir.SyncInfo(
                        on_wait=keep,
                        on_update=list(si.on_update) if si.on_update else [],
                    )
                    changed = True
                out.append(inst)
            if changed:
                bb.instructions = out


# ---------------------------------------------------------------------------
# Device program
# ---------------------------------------------------------------------------


def _build_program(C, H, M, S, passes, bc_fold, zero_bias):
    """Emit the SPMD Bass program. `passes` is a list of
    (slot, tile_offset, n_tiles<=2); every core runs the same program on its
    own data."""
    NKC = C // TILE          # K-tiles over C (8)
    NMH = HCHUNK // TILE     # M-tiles per H-chunk (4)
    NHC = H // HCHUNK        # H-chunks (8)
    NC2 = C // 512           # output column chunks (2)
    HN = H // TILE           # bias columns (32)

    WCOLS = NKC * HCHUNK + NMH * C  # w1-part then w2-part, tile-contiguous

    nc = bass.Bass("TRN2", target_bir_lowering=False, debug=False)
    xc = nc.dram_tensor("xc", [M, C], F32, kind="ExternalInput").ap()
    wr = nc.dram_tensor("wr", [S, NHC, TILE, WCOLS], BF16, kind="ExternalInput").ap()
    wcb = nc.dram_tensor("wcb", [S, TILE, C], BF16, kind="ExternalInput").ap()
    if not zero_bias:
        b1b = nc.dram_tensor("b1b", [S, TILE, HN], F32, kind="ExternalInput").ap()
    yc = nc.dram_tensor("yc", [M, C], F32, kind="ExternalOutput").ap()

    with TileContext(nc) as tc, ExitStack() as ctx:
        cpool = ctx.enter_context(tc.tile_pool(name="const", bufs=1))
        ident = cpool.tile([TILE, TILE], BF16, tag="ident")
        make_identity(nc, ident[:])
        epsc = cpool.tile([TILE, 1], F32, tag="eps")
        nc.gpsimd.memset(epsc[:], LN_EPS)
        bccs = []
        for si in range(S):
            t = cpool.tile([TILE, 1], F32, tag=f"bc{si}")
            nc.gpsimd.memset(t[:], float(bc_fold[si]))
            bccs.append(t)

        wpool = ctx.enter_context(tc.tile_pool(name="w", bufs=4))
        spool = ctx.enter_context(tc.tile_pool(name="slot", bufs=2))
        xpool = ctx.enter_context(tc.tile_pool(name="x", bufs=6))
        hpool = ctx.enter_context(tc.tile_pool(name="h", bufs=3))
        htpool = ctx.enter_context(tc.tile_pool(name="ht", bufs=2))
        kpool = ctx.enter_context(tc.tile_pool(name="kt", bufs=4))
        opool = ctx.enter_context(tc.tile_pool(name="o", bufs=4))
        stpool = ctx.enter_context(tc.tile_pool(name="st", bufs=8))
        sqpool = ctx.enter_context(tc.tile_pool(name="sq", bufs=2))
        pps = ctx.enter_context(tc.tile_pool(name="pk", bufs=2, space="PSUM"))
        ppy = ctx.enter_context(tc.tile_pool(name="py", bufs=4, space="PSUM"))
        ppt = ctx.enter_context(tc.tile_pool(name="ptr", bufs=2, space="PSUM"))

        prev_slot = -1
        wcb_sb = None
        b1_sb = None
        # prefetch the first two weight chunks before any LN work so the
        # first mm1 group is not gated on a cold 2MB weight DMA
        wt_pre = []
        for hc0 in range(2):
            wtp = wpool.tile([TILE, WCOLS], BF16, tag="w", name=f"wtp{hc0}")
            nc.sync.dma_start(wtp[:], wr[passes[0][0], hc0])
            wt_pre.append(wtp)
        for pass_idx, (si, tile_off, nt) in enumerate(passes):
            ntok = TILE * nt
            if si != prev_slot:
                wcb_sb = spool.tile([TILE, C], BF16, tag="wcb")
                nc.sync.dma_start(wcb_sb[:], wcb[si])
                if not zero_bias:
                    b1_sb = spool.tile([TILE, HN], F32, tag="b1")
                    nc.sync.dma_start(b1_sb[:], b1b[si])
                prev_slot = si

            x_t = []
            s_t = []
            hT = htpool.tile([TILE, NKC, ntok], BF16, tag="hT")
            for t in range(nt):
                row0 = (tile_off + t) * TILE
                xt = xpool.tile([TILE, C], F32, tag="x")
                x_t.append(xt)
                nc.sync.dma_start(xt[:], xc[row0 : row0 + TILE, :])

                nsum = stpool.tile([TILE, 1], F32, tag="nsum")
                nc.vector.reduce_sum(
                    nsum[:], xt[:], axis=mybir.AxisListType.X, negate=True
                )
                negmu = stpool.tile([TILE, 1], F32, tag="negmu")
                nc.scalar.mul(negmu[:], nsum[:], 1.0 / C)
                sq = sqpool.tile([TILE, C], F32, tag="sq")
                ssq = stpool.tile([TILE, 1], F32, tag="ssq")
                nc.scalar.activation(
                    sq[:], xt[:], AF.Square, bias=negmu[:], scale=1.0,
                    accum_out=ssq[:],
                )
                std = stpool.tile([TILE, 1], F32, tag="std")
                nc.scalar.activation(
                    std[:], ssq[:], AF.Sqrt, bias=epsc[:], scale=1.0 / C
                )
                rs = stpool.tile([TILE, 1], F32, tag="rs")
                nc.vector.reciprocal(rs[:], std[:])
                nmrs = stpool.tile([TILE, 1], F32, tag="nmrs")
                nc.vector.tensor_mul(nmrs[:], negmu[:], rs[:])
                ht_ = hpool.tile([TILE, C], BF16, tag="h")
                nc.scalar.activation(
                    ht_[:], xt[:], AF.Identity, bias=nmrs[:], scale=rs[:]
                )

                # confidence -> straight-through scale
                prod = hpool.tile([TILE, C], BF16, tag="prod")
                cdot = stpool.tile([TILE, 1], F32, tag="cdot")
                nc.vector.scalar_tensor_tensor(
                    prod[:], ht_[:], 1.0, wcb_sb[:], op0=OP.mult, op1=OP.mult,
                    accum_out=cdot[:],
                )
                conf = stpool.tile([TILE, 1], F32, tag="conf")
                nc.scalar.activation(
                    conf[:], cdot[:], AF.Sigmoid, bias=bccs[si][:], scale=1.0
                )
                cpe = stpool.tile([TILE, 1], F32, tag="cpe")
                nc.vector.tensor_scalar_add(cpe[:], conf[:], 1e-6)
                rc = stpool.tile([TILE, 1], F32, tag="rc")
                nc.vector.reciprocal(rc[:], cpe[:])
                sc = stpool.tile([TILE, 1], F32, tag="sc")
                nc.vector.tensor_mul(sc[:], conf[:], rc[:])
                s_t.append(sc)

                # h^T tiles for the matmuls
                for kc in range(NKC):
                    pt = ppt.tile([TILE, TILE], BF16, tag="ptr")
                    nc.tensor.transpose(
                        pt[:], ht_[:, kc * TILE : (kc + 1) * TILE], ident[:]
                    )
                    nc.vector.tensor_copy(
                        hT[:, kc, t * TILE : (t + 1) * TILE], pt[:]
                    )

            ys = [
                ppy.tile([TILE, 512], F32, tag="py", name=f"ys{i}")
                for i in range(nt * NC2)
            ]

            def emit_mm2(hc, mh, kt, wt):
                w2base = NKC * HCHUNK + mh * C
                for t in range(nt):
                    for ncx in range(NC2):
                        nc.tensor.matmul(
                            ys[t * NC2 + ncx][:],
                            kt[:, t * TILE : (t + 1) * TILE],
                            wt[:, w2base + ncx * 512 : w2base + (ncx + 1) * 512],
                            start=(hc == 0 and mh == 0),
                            stop=(hc == NHC - 1 and mh == NMH - 1),
                        )

            # mm2(mh) depends on the ACT+DVE relu^2 of mm1(mh)'s psum; the PE
            # is in-order, so emit mm1(mh+1) before mm2(mh) to hide that
            # latency behind a full mm1 group.
            pending = None  # (hc, mh, kt, wt)
            for hc in range(NHC):
                if pass_idx == 0 and hc < len(wt_pre):
                    wt = wt_pre[hc]
                else:
                    wt = wpool.tile([TILE, WCOLS], BF16, tag="w")
                    nc.sync.dma_start(wt[:], wr[si, hc])
                for mh in range(NMH):
                    pk = pps.tile([TILE, ntok], F32, tag="pk")
                    for kc in range(NKC):
                        nc.tensor.matmul(
                            pk[:],
                            wt[:, kc * HCHUNK + mh * TILE : kc * HCHUNK + (mh + 1) * TILE],
                            hT[:, kc, :],
                            start=(kc == 0),
                            stop=(kc == NKC - 1),
                        )
                    if pending is not None:
                        emit_mm2(*pending)
                    kt = kpool.tile([TILE, ntok], BF16, tag="kt")
                    kr = kpool.tile([TILE, ntok], BF16, tag="kr")
                    if zero_bias:
                        bias_ap = 0.0
                    else:
                        col = hc * NMH + mh
                        bias_ap = b1_sb[:, col : col + 1]
                    nc.scalar.activation(
                        kr[:], pk[:], AF.Relu, bias=bias_ap, scale=1.0
                    )
                    nc.vector.tensor_mul(kt[:], kr[:], kr[:])
                    pending = (hc, mh, kt, wt)
            emit_mm2(*pending)
            for t in range(nt):
                row0 = (tile_off + t) * TILE
                ot = opool.tile([TILE, C], F32, tag="o")
                for ncx in range(NC2):
                    nc.vector.scalar_tensor_tensor(
                        ot[:, ncx * 512 : (ncx + 1) * 512],
                        ys[t * NC2 + ncx][:],
                        s_t[t][:],
                        x_t[t][:, ncx * 512 : (ncx + 1) * 512],
                        op0=OP.mult,
                        op1=OP.add,
                    )
                nc.sync.dma_start(yc[row0 : row0 + TILE, :], ot[:])

    _split_excess_waits(nc, 1)
    return nc


# ---------------------------------------------------------------------------
# Host-side dispatch
# ---------------------------------------------------------------------------


def _prepare(x, winners, gamma, beta, w1, w2, wc, bc):
    x = np.ascontiguousarray(np.asarray(x, dtype=np.float32))
    winners = np.asarray(winners).reshape(-1).astype(np.int64)
    gamma = np.asarray(gamma, dtype=np.float32)
    beta = np.asarray(beta, dtype=np.float32)
    w1 = np.asarray(w1, dtype=np.float32)
    w2 = np.asarray(w2, dtype=np.float32)
    wc = np.asarray(wc, dtype=np.float32)
    bc = np.asarray(bc, dtype=np.float32)

    B, T, C = x.shape
    E, _, H = w1.shape
    N = B * T
    xf = x.reshape(N, C)

    order = np.argsort(winners, kind="stable")
    counts = np.bincount(winners, minlength=E)

    slots = [e for e in range(E) if counts[e] > 0]
    S = len(slots)
    grain = TILE * NCORES

    per_core_idx = [[] for _ in range(NCORES)]
    passes = []
    pos = 0
    tile_off = 0
    for si, e in enumerate(slots):
        n_e = int(counts[e])
        m_e = int(math.ceil(n_e / grain))
        padded = np.full(m_e * grain, -1, dtype=np.int64)
        padded[:n_e] = order[pos : pos + n_e]
        pos += n_e
        resh = padded.reshape(m_e, NCORES, TILE)
        for c in range(NCORES):
            per_core_idx[c].append(resh[:, c, :].reshape(-1))
        j = 0
        while j < m_e:
            nt = min(2, m_e - j)
            passes.append((si, tile_off + j, nt))
            j += nt
        tile_off += m_e
    per_core_idx = [np.concatenate(lst) for lst in per_core_idx]
    M = per_core_idx[0].size

    # fold gamma/beta
    NKC = C // TILE
    NMH = HCHUNK // TILE
    NHC = H // HCHUNK
    w1f = (w1[slots] * gamma[None, :, None]).astype(NP_BF16)
    w2f = w2[slots].astype(NP_BF16)
    # re-layout weights into the exact SBUF tile order so each (slot, hchunk)
    # is ONE contiguous [128, WCOLS] DMA (16KB per partition row)
    w1part = (
        w1f.reshape(S, NKC, TILE, NHC, HCHUNK)
        .transpose(0, 3, 2, 1, 4)
        .reshape(S, NHC, TILE, NKC * HCHUNK)
    )
    w2part = (
        w2f.reshape(S, NHC, NMH, TILE, C)
        .transpose(0, 1, 3, 2, 4)
        .reshape(S, NHC, TILE, NMH * C)
    )
    wrearr = np.ascontiguousarray(np.concatenate([w1part, w2part], axis=3))
    wcf = (wc[slots] * gamma[None, :]).astype(NP_BF16)
    wcb = np.ascontiguousarray(
        np.broadcast_to(wcf[:, None, :], (S, TILE, C))
    )
    zero_bias = bool(np.all(beta == 0.0))
    bc_fold = [float(bc[e] + float(beta @ wc[e])) for e in slots]
    b1b = None
    if not zero_bias:
        b1 = np.einsum("c,sch->sh", beta, w1[slots])
        b1b = np.ascontiguousarray(
            b1.reshape(S, H // TILE, TILE).transpose(0, 2, 1)
        ).astype(np.float32)

    in_maps = []
    for c in range(NCORES):
        idx = per_core_idx[c]
        xcrows = np.zeros((M, C), dtype=np.float32)
        valid = idx >= 0
        xcrows[valid] = xf[idx[valid]]
        m = {"xc": xcrows, "wr": wrearr, "wcb": wcb}
        if not zero_bias:
            m["b1b"] = b1b
        in_maps.append(m)

    meta = dict(
        B=B, T=T, C=C, H=H, N=N, M=M, S=S, passes=passes,
        bc_fold=bc_fold, zero_bias=zero_bias, per_core_idx=per_core_idx,
        xf=xf,
    )
    return in_maps, meta


def _assemble(results, meta):
    N, C = meta["N"], meta["C"]
    out = np.empty((N, C), dtype=np.float32)
    seen = np.zeros(N, dtype=bool)
    for c in range(NCORES):
        idx = meta["per_core_idx"][c]
        valid = idx >= 0
        out[idx[valid]] = results[c]["yc"][valid]
        seen[idx[valid]] = True
    assert seen.all()
    return out.reshape(meta["B"], meta["T"], C)


def kernel_with_results(x, winners, gamma, beta, w1, w2, wc, bc, **run_kwargs):
    in_maps, meta = _prepare(x, winners, gamma, beta, w1, w2, wc, bc)
    nc = _build_program(
        meta["C"], meta["H"], meta["M"], meta["S"], meta["passes"],
        meta["bc_fold"], meta["zero_bias"],
    )
    res = run_bass_kernel_spmd(nc, in_maps, core_ids=list(range(NCORES)), **run_kwargs)
    return _assemble(res.results, meta), res


def kernel(x, winners, gamma, beta, w1, w2, wc, bc):
    out, _ = kernel_with_results(x, winners, gamma, beta, w1, w2, wc, bc)
    return out


# revision 15
# speedup vs baseline: 1.1567x; 1.0974x over previous
"""CaMoE block (LayerNorm -> per-expert squared-ReLU FFN with top-1 routing,
confidence-scaled combine, residual) on 8 Trainium2 NeuronCores.

Strategy (token-parallel with expert-grouped tiles):
  * Host: stable-sort tokens by winning expert, pad each expert group to a
    multiple of 128*8 so every core receives the SAME number of 128-token
    tiles per expert. This makes the SPMD program identical across cores
    while every 128-token tile has a single expert.
  * Device (per core): for each 128-token tile: LayerNorm (token-major),
    confidence sigmoid(h.wc+bc) and straight-through scale c/(c+1e-6);
    transpose h via the PE; then stream the expert's W1/W2 in H-chunks and
    run  kT = relu(W1^T h^T)^2  (PE + DVE) and  y += kT^T W2chunk  (PE),
    finally  out = y*scale + x  (DVE) and DMA out.
  * Host: scatter rows back to their original token positions.

gamma/beta of the LayerNorm are folded into W1/wc on the host (plus an
additive H-bias when beta != 0), so the device computes the pre-affine LN.
All matmuls run in bf16 with fp32 PSUM accumulation.
"""

import math
import os
from contextlib import ExitStack

import numpy as np

import concourse.bass as bass
import concourse.mybir as mybir
import concourse.tile as tile
from concourse.bass_utils import run_bass_kernel_spmd
from concourse.masks import make_identity
from concourse.tile import TileContext, ScopedClock

AF = mybir.ActivationFunctionType
OP = mybir.AluOpType
BF16 = mybir.dt.bfloat16
F32 = mybir.dt.float32
NP_BF16 = mybir.dt.np(BF16)

NCORES = 8
TILE = 128
HCHUNK = 512
LN_EPS = 1e-5

# ---------------------------------------------------------------------------
# Workarounds for the walrus build in this environment: it encodes at most
# ONE semaphore wait per instruction and cannot split multi-wait
# instructions itself ("Too many sync wait commands"). We (a) emit the
# TileContext tail-drain waits one-per-NoOp and (b) post-process the whole
# program to hoist excess waits onto same-engine NoOps.
# ---------------------------------------------------------------------------


def _patched_drain_and_barrier(self, tick_clock, wait_clock):
    probe = self.nc.sync.nop(nofuse=True)
    wait_clock.add_sem_waits(probe.ins, ScopedClock({None: tick_clock.global_clock}))
    si = probe.ins.sync_info
    waits = list(si.on_wait) if si is not None and si.on_wait else []
    if len(waits) > 1:
        probe.ins.sync_info = mybir.SyncInfo(on_wait=[waits[0]], on_update=[])
        for w in waits[1:]:
            n = self.nc.sync.nop(nofuse=True)
            n.ins.sync_info = mybir.SyncInfo(on_wait=[w], on_update=[])
    self.nc.sync.drain()
    self.nc.all_engine_barrier()
    assert self.sems is not None
    popped = self.nc._tile_sem_poison_stack.pop()
    assert popped is self._sem_poison
    self.nc.clear_and_free_semaphores(list(self.sems.allocated().values()))
    self.nc.all_engine_barrier()


TileContext._drain_and_barrier = _patched_drain_and_barrier


def _split_excess_waits(nc, max_waits: int = 1):
    for fn in nc.m.functions:
        for bb in fn.blocks:
            insts = list(bb.instructions)
            out = []
            changed = False
            for inst in insts:
                si = inst.sync_info
                waits = list(si.on_wait) if si is not None and si.on_wait else []
                if len(waits) > max_waits:
                    extra = waits[:-max_waits]
                    keep = waits[-max_waits:]
                    for j, w in enumerate(extra):
                        nop = mybir.InstNoOp(
                            name=f"{inst.name}-wsplit{j}", ins=[], outs=[]
                        )
                        nop.engine = inst.engine
                        nop.sync_info = mybir.SyncInfo(on_wait=[w], on_update=[])
                        out.append(nop)
                    inst.sync_info = mybir.SyncInfo(
                        on_wait=keep,
                        on_update=list(si.on_update) if si.on_update else [],
                    )
                    changed = True
                out.append(inst)
            if changed:
                bb.instructions = out


# ---------------------------------------------------------------------------
# Device program
# ---------------------------------------------------------------------------


def _build_program(C, H, M, S, passes, bc_fold, zero_bias):
    """Emit the SPMD Bass program. `passes` is a list of
    (slot, tile_offset, n_tiles<=2); every core runs the same program on its
    own data."""
    NKC = C // TILE          # K-tiles over C (8)
    NMH = HCHUNK // TILE     # M-tiles per H-chunk (4)
    NHC = H // HCHUNK        # H-chunks (8)
    NC2 = C // 512           # output column chunks (2)
    HN = H // TILE           # bias columns (32)

    WCOLS = NKC * HCHUNK + NMH * C  # w1-part then w2-part, tile-contiguous

    nc = bass.Bass("TRN2", target_bir_lowering=False, debug=False)
    xc = nc.dram_tensor("xc", [M, C], F32, kind="ExternalInput").ap()
    wr = nc.dram_tensor("wr", [S, NHC, TILE, WCOLS], BF16, kind="ExternalInput").ap()
    wcb = nc.dram_tensor("wcb", [S, TILE, C], BF16, kind="ExternalInput").ap()
    if not zero_bias:
        b1b = nc.dram_tensor("b1b", [S, TILE, HN], F32, kind="ExternalInput").ap()
    yc = nc.dram_tensor("yc", [M, C], F32, kind="ExternalOutput").ap()

    with TileContext(nc) as tc, ExitStack() as ctx:
        cpool = ctx.enter_context(tc.tile_pool(name="const", bufs=1))
        ident = cpool.tile([TILE, TILE], BF16, tag="ident")
        make_identity(nc, ident[:])
        epsc = cpool.tile([TILE, 1], F32, tag="eps")
        nc.gpsimd.memset(epsc[:], LN_EPS)
        bccs = []
        for si in range(S):
            t = cpool.tile([TILE, 1], F32, tag=f"bc{si}")
            nc.gpsimd.memset(t[:], float(bc_fold[si]))
            bccs.append(t)

        wpool = ctx.enter_context(tc.tile_pool(name="w", bufs=4))
        spool = ctx.enter_context(tc.tile_pool(name="slot", bufs=2))
        xpool = ctx.enter_context(tc.tile_pool(name="x", bufs=6))
        hpool = ctx.enter_context(tc.tile_pool(name="h", bufs=3))
        htpool = ctx.enter_context(tc.tile_pool(name="ht", bufs=2))
        kpool = ctx.enter_context(tc.tile_pool(name="kt", bufs=4))
        opool = ctx.enter_context(tc.tile_pool(name="o", bufs=4))
        stpool = ctx.enter_context(tc.tile_pool(name="st", bufs=8))
        sqpool = ctx.enter_context(tc.tile_pool(name="sq", bufs=2))
        pps = ctx.enter_context(tc.tile_pool(name="pk", bufs=2, space="PSUM"))
        ppy = ctx.enter_context(tc.tile_pool(name="py", bufs=4, space="PSUM"))
        ppt = ctx.enter_context(tc.tile_pool(name="ptr", bufs=2, space="PSUM"))

        slot_consts = {}

        def get_slot_consts(si):
            # NOTE: spool bufs must cover the number of distinct slots alive
            # at once (current + next pass's). Entries are invalidated by the
            # pool's slot reuse; with bufs=2 and passes grouped by slot this
            # holds.
            if si not in slot_consts:
                wcb_sb = spool.tile([TILE, C], BF16, tag="wcb", name=f"wcb{si}")
                nc.sync.dma_start(wcb_sb[:], wcb[si])
                b1_sb = None
                if not zero_bias:
                    b1_sb = spool.tile([TILE, HN], F32, tag="b1", name=f"b1{si}")
                    nc.sync.dma_start(b1_sb[:], b1b[si])
                slot_consts[si] = (wcb_sb, b1_sb)
            return slot_consts[si]

        def emit_stage_a(pass_idx):
            si, tile_off, nt = passes[pass_idx]
            ntok = TILE * nt
            wcb_sb, _ = get_slot_consts(si)
            x_t = []
            s_t = []
            hT = htpool.tile([TILE, NKC, ntok], BF16, tag="hT",
                             name=f"hT{pass_idx}")
            for t in range(nt):
                row0 = (tile_off + t) * TILE
                xt = xpool.tile([TILE, C], F32, tag="x", name=f"x{pass_idx}_{t}")
                x_t.append(xt)
                nc.sync.dma_start(xt[:], xc[row0 : row0 + TILE, :])

                nsum = stpool.tile([TILE, 1], F32, tag="nsum")
                nc.vector.reduce_sum(
                    nsum[:], xt[:], axis=mybir.AxisListType.X, negate=True
                )
                negmu = stpool.tile([TILE, 1], F32, tag="negmu")
                nc.scalar.mul(negmu[:], nsum[:], 1.0 / C)
                sq = sqpool.tile([TILE, C], F32, tag="sq")
                ssq = stpool.tile([TILE, 1], F32, tag="ssq")
                nc.scalar.activation(
                    sq[:], xt[:], AF.Square, bias=negmu[:], scale=1.0,
                    accum_out=ssq[:],
                )
                std = stpool.tile([TILE, 1], F32, tag="std")
                nc.scalar.activation(
                    std[:], ssq[:], AF.Sqrt, bias=epsc[:], scale=1.0 / C
                )
                rs = stpool.tile([TILE, 1], F32, tag="rs")
                nc.vector.reciprocal(rs[:], std[:])
                nmrs = stpool.tile([TILE, 1], F32, tag="nmrs")
                nc.vector.tensor_mul(nmrs[:], negmu[:], rs[:])
                ht_ = hpool.tile([TILE, C], BF16, tag="h")
                nc.scalar.activation(
                    ht_[:], xt[:], AF.Identity, bias=nmrs[:], scale=rs[:]
                )

                # confidence -> straight-through scale
                prod = hpool.tile([TILE, C], BF16, tag="prod")
                cdot = stpool.tile([TILE, 1], F32, tag="cdot")
                nc.vector.scalar_tensor_tensor(
                    prod[:], ht_[:], 1.0, wcb_sb[:], op0=OP.mult, op1=OP.mult,
                    accum_out=cdot[:],
                )
                conf = stpool.tile([TILE, 1], F32, tag="conf")
                nc.scalar.activation(
                    conf[:], cdot[:], AF.Sigmoid, bias=bccs[si][:], scale=1.0
                )
                cpe = stpool.tile([TILE, 1], F32, tag="cpe")
                nc.vector.tensor_scalar_add(cpe[:], conf[:], 1e-6)
                rc = stpool.tile([TILE, 1], F32, tag="rc")
                nc.vector.reciprocal(rc[:], cpe[:])
                sc = stpool.tile([TILE, 1], F32, tag="sc")
                nc.vector.tensor_mul(sc[:], conf[:], rc[:])
                s_t.append(sc)

                # h^T tiles for the matmuls
                for kc in range(NKC):
                    pt = ppt.tile([TILE, TILE], BF16, tag="ptr")
                    nc.tensor.transpose(
                        pt[:], ht_[:, kc * TILE : (kc + 1) * TILE], ident[:]
                    )
                    nc.vector.tensor_copy(
                        hT[:, kc, t * TILE : (t + 1) * TILE], pt[:]
                    )
            return x_t, s_t, hT

        stage_a = {}

        def get_stage_a(pass_idx):
            if pass_idx < len(passes) and pass_idx not in stage_a:
                stage_a[pass_idx] = emit_stage_a(pass_idx)
            return stage_a.get(pass_idx)

        for pass_idx, (si, tile_off, nt) in enumerate(passes):
            ntok = TILE * nt
            x_t, s_t, hT = get_stage_a(pass_idx)
            _, b1_sb = get_slot_consts(si)

            ys = [
                ppy.tile([TILE, 512], F32, tag="py", name=f"ys{i}")
                for i in range(nt * NC2)
            ]

            def emit_mm2(hc, mh, kt, wt):
                w2base = NKC * HCHUNK + mh * C
                for t in range(nt):
                    for ncx in range(NC2):
                        nc.tensor.matmul(
                            ys[t * NC2 + ncx][:],
                            kt[:, t * TILE : (t + 1) * TILE],
                            wt[:, w2base + ncx * 512 : w2base + (ncx + 1) * 512],
                            start=(hc == 0 and mh == 0),
                            stop=(hc == NHC - 1 and mh == NMH - 1),
                        )

            # mm2(mh) depends on the ACT+DVE relu^2 of mm1(mh)'s psum; the PE
            # is in-order, so emit mm1(mh+1) before mm2(mh) to hide that
            # latency behind a full mm1 group.
            pending = None  # (hc, mh, kt, wt)
            for hc in range(NHC):
                wt = wpool.tile([TILE, WCOLS], BF16, tag="w")
                nc.sync.dma_start(wt[:], wr[si, hc])
                if hc == 2:
                    # emit the next pass's LayerNorm/conf/transposes here so
                    # the in-order ACT/DVE streams reach them mid-pass
                    # instead of after all of this pass's relu ops
                    get_stage_a(pass_idx + 1)
                for mh in range(NMH):
                    pk = pps.tile([TILE, ntok], F32, tag="pk")
                    for kc in range(NKC):
                        nc.tensor.matmul(
                            pk[:],
                            wt[:, kc * HCHUNK + mh * TILE : kc * HCHUNK + (mh + 1) * TILE],
                            hT[:, kc, :],
                            start=(kc == 0),
                            stop=(kc == NKC - 1),
                        )
                    if pending is not None:
                        emit_mm2(*pending)
                    kt = kpool.tile([TILE, ntok], BF16, tag="kt")
                    kr = kpool.tile([TILE, ntok], BF16, tag="kr")
                    if zero_bias:
                        bias_ap = 0.0
                    else:
                        col = hc * NMH + mh
                        bias_ap = b1_sb[:, col : col + 1]
                    nc.scalar.activation(
                        kr[:], pk[:], AF.Relu, bias=bias_ap, scale=1.0
                    )
                    nc.vector.tensor_mul(kt[:], kr[:], kr[:])
                    pending = (hc, mh, kt, wt)
            emit_mm2(*pending)
            for t in range(nt):
                row0 = (tile_off + t) * TILE
                ot = opool.tile([TILE, C], F32, tag="o")
                for ncx in range(NC2):
                    nc.vector.scalar_tensor_tensor(
                        ot[:, ncx * 512 : (ncx + 1) * 512],
                        ys[t * NC2 + ncx][:],
                        s_t[t][:],
                        x_t[t][:, ncx * 512 : (ncx + 1) * 512],
                        op0=OP.mult,
                        op1=OP.add,
                    )
                nc.sync.dma_start(yc[row0 : row0 + TILE, :], ot[:])

    _split_excess_waits(nc, 1)
    return nc


# ---------------------------------------------------------------------------
# Host-side dispatch
# ---------------------------------------------------------------------------


def _prepare(x, winners, gamma, beta, w1, w2, wc, bc):
    x = np.ascontiguousarray(np.asarray(x, dtype=np.float32))
    winners = np.asarray(winners).reshape(-1).astype(np.int64)
    gamma = np.asarray(gamma, dtype=np.float32)
    beta = np.asarray(beta, dtype=np.float32)
    w1 = np.asarray(w1, dtype=np.float32)
    w2 = np.asarray(w2, dtype=np.float32)
    wc = np.asarray(wc, dtype=np.float32)
    bc = np.asarray(bc, dtype=np.float32)

    B, T, C = x.shape
    E, _, H = w1.shape
    N = B * T
    xf = x.reshape(N, C)

    order = np.argsort(winners, kind="stable")
    counts = np.bincount(winners, minlength=E)

    slots = [e for e in range(E) if counts[e] > 0]
    S = len(slots)
    grain = TILE * NCORES

    per_core_idx = [[] for _ in range(NCORES)]
    passes = []
    pos = 0
    tile_off = 0
    for si, e in enumerate(slots):
        n_e = int(counts[e])
        m_e = int(math.ceil(n_e / grain))
        padded = np.full(m_e * grain, -1, dtype=np.int64)
        padded[:n_e] = order[pos : pos + n_e]
        pos += n_e
        resh = padded.reshape(m_e, NCORES, TILE)
        for c in range(NCORES):
            per_core_idx[c].append(resh[:, c, :].reshape(-1))
        j = 0
        while j < m_e:
            nt = min(2, m_e - j)
            passes.append((si, tile_off + j, nt))
            j += nt
        tile_off += m_e
    per_core_idx = [np.concatenate(lst) for lst in per_core_idx]
    M = per_core_idx[0].size

    # fold gamma/beta
    NKC = C // TILE
    NMH = HCHUNK // TILE
    NHC = H // HCHUNK
    w1f = (w1[slots] * gamma[None, :, None]).astype(NP_BF16)
    w2f = w2[slots].astype(NP_BF16)
    # re-layout weights into the exact SBUF tile order so each (slot, hchunk)
    # is ONE contiguous [128, WCOLS] DMA (16KB per partition row)
    w1part = (
        w1f.reshape(S, NKC, TILE, NHC, HCHUNK)
        .transpose(0, 3, 2, 1, 4)
        .reshape(S, NHC, TILE, NKC * HCHUNK)
    )
    w2part = (
        w2f.reshape(S, NHC, NMH, TILE, C)
        .transpose(0, 1, 3, 2, 4)
        .reshape(S, NHC, TILE, NMH * C)
    )
    wrearr = np.ascontiguousarray(np.concatenate([w1part, w2part], axis=3))
    wcf = (wc[slots] * gamma[None, :]).astype(NP_BF16)
    wcb = np.ascontiguousarray(
        np.broadcast_to(wcf[:, None, :], (S, TILE, C))
    )
    zero_bias = bool(np.all(beta == 0.0))
    bc_fold = [float(bc[e] + float(beta @ wc[e])) for e in slots]
    b1b = None
    if not zero_bias:
        b1 = np.einsum("c,sch->sh", beta, w1[slots])
        b1b = np.ascontiguousarray(
            b1.reshape(S, H // TILE, TILE).transpose(0, 2, 1)
        ).astype(np.float32)

    in_maps = []
    for c in range(NCORES):
        idx = per_core_idx[c]
        xcrows = np.zeros((M, C), dtype=np.float32)
        valid = idx >= 0
        xcrows[valid] = xf[idx[valid]]
        m = {"xc": xcrows, "wr": wrearr, "wcb": wcb}
        if not zero_bias:
            m["b1b"] = b1b
        in_maps.append(m)

    meta = dict(
        B=B, T=T, C=C, H=H, N=N, M=M, S=S, passes=passes,
        bc_fold=bc_fold, zero_bias=zero_bias, per_core_idx=per_core_idx,
        xf=xf,
    )
    return in_maps, meta


def _assemble(results, meta):
    N, C = meta["N"], meta["C"]
    out = np.empty((N, C), dtype=np.float32)
    seen = np.zeros(N, dtype=bool)
    for c in range(NCORES):
        idx = meta["per_core_idx"][c]
        valid = idx >= 0
        out[idx[valid]] = results[c]["yc"][valid]
        seen[idx[valid]] = True
    assert seen.all()
    return out.reshape(meta["B"], meta["T"], C)


def kernel_with_results(x, winners, gamma, beta, w1, w2, wc, bc, **run_kwargs):
    in_maps, meta = _prepare(x, winners, gamma, beta, w1, w2, wc, bc)
    nc = _build_program(
        meta["C"], meta["H"], meta["M"], meta["S"], meta["passes"],
        meta["bc_fold"], meta["zero_bias"],
    )
    res = run_bass_kernel_spmd(nc, in_maps, core_ids=list(range(NCORES)), **run_kwargs)
    return _assemble(res.results, meta), res


def kernel(x, winners, gamma, beta, w1, w2, wc, bc):
    out, _ = kernel_with_results(x, winners, gamma, beta, w1, w2, wc, bc)
    return out
